# revision 3
# baseline (speedup 1.0000x reference)
"""Trainium2 Bass kernel for nn_LipschitzNet (8-core SPMD, batch-sharded).

Math (reference, with beta=0.75, gamma=0.01, dt=1e-3):
    A = M_A - 0.5*M_A.T - 0.01*I        W = M_W - 0.5*M_W.T - 0.01*I
    Z[t,d,h] = sum_b x[b,t,d] * E_w[h,b] + E_b[h]
    h_{t+1} = h_t + dt*(h_t @ A) + dt*tanh(h_t @ W + Z[t])
    out = h_T @ D_w.T + D_b

Rescaled recurrence used on device (H = h/dt, exact in infinite precision):
    P  = I + dt*A          Wd = dt*W
    H_{t+1} = H_t @ P + tanh(H_t @ Wd + Z[t])
    out = dt * (H_T @ D_w.T) + D_b

Sharding: batch rows (= the d index of x via the module's transpose-based
math) are split 16 per core; each core runs the full sequential scan on its
shard with replicated weights and writes its [16, 24] slice of the output.

Matmuls run in float32r (12-mantissa-bit fp32, full PE rate for N>=256).
The state is kept transposed (G = H.T as 8 [128,16] k-tiles) so it can be
the matmul stationary; each step's new H (batch-major, from PSUM) is
re-transposed on the PE.
"""
import numpy as np

import concourse.bass as bass
import concourse.tile as tile
from concourse import bacc, mybir
from concourse.bass_utils import run_bass_kernel_spmd
from concourse.masks import make_identity

FP32 = mybir.dt.float32
FP32R = mybir.dt.float32r
AF = mybir.ActivationFunctionType
ALU = mybir.AluOpType

HID = 1024
B = 128
T = 512
OUT = 24
DT = 0.001
NCORES = 8
BS = B // NCORES  # 16 batch rows per core
KT = HID // 128  # 8 k-tiles


def _scaled_identity(nc, ap, c):
    nc.gpsimd.memset(ap, 0.0)
    nc.gpsimd.affine_select(
        out=ap,
        in_=ap,
        compare_op=ALU.not_equal,
        fill=c,
        base=0,
        pattern=[[-1, ap.shape[0]]],
        channel_multiplier=1,
    )


def build(t_steps=T):
    from contextlib import ExitStack

    assert t_steps % 8 == 0
    nc = bacc.Bacc("TRN2")
    xs = nc.dram_tensor("xs", [B, t_steps, BS], FP32, kind="ExternalInput")
    MA = nc.dram_tensor("MA", [HID, HID], FP32, kind="ExternalInput")
    MW = nc.dram_tensor("MW", [HID, HID], FP32, kind="ExternalInput")
    Ewt = nc.dram_tensor("Ewt", [B, HID], FP32, kind="ExternalInput")
    Ebb = nc.dram_tensor("Ebb", [B, HID], FP32, kind="ExternalInput")
    Dwt = nc.dram_tensor("Dwt", [HID, OUT], FP32, kind="ExternalInput")
    Dbb = nc.dram_tensor("Dbb", [B, OUT], FP32, kind="ExternalInput")
    out = nc.dram_tensor("out", [BS, OUT], FP32, kind="ExternalOutput")
    Zd = nc.dram_tensor("Zd", [t_steps, BS, HID], FP32)

    with tile.TileContext(nc) as tc, ExitStack() as ctx:
        consts = ctx.enter_context(tc.tile_pool(name="consts", bufs=1))
        ident = consts.tile([128, 128], FP32)
        make_identity(nc, ident[:])
        identP = consts.tile([128, 128], FP32)  # (1 - 0.01*dt) * I
        _scaled_identity(nc, identP[:], 1.0 - 0.01 * DT)
        identW = consts.tile([128, 128], FP32)  # -0.01*dt * I
        _scaled_identity(nc, identW[:], -0.01 * DT)
        Ebb_sb = consts.tile([128, HID], FP32)
        nc.sync.dma_start(Ebb_sb[:], Ebb[:])
        Dbb_sb = consts.tile([128, OUT], FP32)
        nc.sync.dma_start(Dbb_sb[:], Dbb[:])
        Ewt_r = consts.tile([128, HID], FP32R)
        nc.gpsimd.dma_start(Ewt_r[:], Ewt[:])  # cast fp32 -> fp32r
        Dwt_r = consts.tile([128, KT * OUT], FP32R)
        nc.gpsimd.dma_start(Dwt_r[:], Dwt[:].rearrange("(k p) o -> p k o", p=128))
        P_r = consts.tile([128, KT * HID], FP32R)
        Wd_r = consts.tile([128, KT * HID], FP32R)

        # ---- weight prep: P = I + dt*(M - 0.5*M.T - 0.01*I), Wd analog ----
        with (
            tc.tile_pool(name="prep", bufs=1) as prep,
            tc.tile_pool(name="prep_ps", bufs=2, space="PSUM") as pps,
        ):
            for M, dst, identD in ((MA, P_r, identP), (MW, Wd_r, identW)):
                stage = prep.tile([128, KT * HID], FP32, tag="stage")
                nc.sync.dma_start(
                    stage[:], M[:].rearrange("(k p) n -> p k n", p=128)
                )
                stageT = prep.tile([128, KT * HID], FP32, tag="stageT")
                for k in range(KT):
                    for j in range(KT):
                        trp = pps.tile([128, 128], FP32)
                        nc.tensor.transpose(
                            trp[:],
                            stage[:, j * HID + 128 * k : j * HID + 128 * (k + 1)],
                            ident[:],
                        )
                        nc.vector.tensor_copy(
                            stageT[:, k * HID + 128 * j : k * HID + 128 * (j + 1)],
                            trp[:],
                        )
                t1 = prep.tile([128, KT * HID], FP32, tag="t1")
                nc.vector.scalar_tensor_tensor(
                    t1[:], stageT[:], -0.5, stage[:], ALU.mult, ALU.add
                )
                nc.vector.tensor_scalar_mul(dst[:], t1[:], DT)
                for k in range(KT):
                    sl = slice(k * HID + 128 * k, k * HID + 128 * (k + 1))
                    nc.vector.scalar_tensor_tensor(
                        dst[:, sl], t1[:, sl], DT, identD[:], ALU.mult, ALU.add
                    )

        # ---- Z = x^T E^T + E_b, stored to DRAM as [t, d, h] ----
        with (
            tc.tile_pool(name="zx", bufs=3) as zx,
            tc.tile_pool(name="zstage", bufs=3) as zs,
            tc.tile_pool(name="zps", bufs=2, space="PSUM") as zp,
        ):
            for o in range(t_steps // 8):
                xr = zx.tile([128, 128], FP32R)
                nc.gpsimd.dma_start(xr[:], xs[:, 8 * o : 8 * o + 8, :])
                ps = zp.tile([128, HID], FP32)
                for h in range(2):
                    nc.tensor.matmul(
                        ps[:, 512 * h : 512 * (h + 1)],
                        xr[:],
                        Ewt_r[:, 512 * h : 512 * (h + 1)],
                        start=True,
                        stop=True,
                    )
                zst = zs.tile([128, HID], FP32)
                nc.vector.tensor_add(zst[:], ps[:], Ebb_sb[:])
                nc.sync.dma_start(Zd[8 * o : 8 * o + 8], zst[:])

        # ---- the sequential scan ----
        with (
            tc.tile_pool(name="g", bufs=2) as gp,
            tc.tile_pool(name="zt", bufs=4) as ztp,
            tc.tile_pool(name="u", bufs=2) as up,
            tc.tile_pool(name="s", bufs=2) as spp,
            tc.tile_pool(name="hn", bufs=2) as hnp,
            tc.tile_pool(name="mm", bufs=1, space="PSUM") as mmp,
            tc.tile_pool(name="tr", bufs=2, space="PSUM") as trpp,
        ):
            g0f = gp.tile([128, 128], FP32, tag="g0f")
            nc.gpsimd.memset(g0f[:], 0.0)
            G = gp.tile([128, 128], FP32R)
            nc.vector.tensor_copy(G[:], g0f[:])
            for t in range(t_steps):
                zt = ztp.tile([BS, HID], FP32)
                nc.sync.dma_start(zt[:], Zd[t])
                hw = mmp.tile([BS, HID], FP32, tag="hw")
                hp = mmp.tile([BS, HID], FP32, tag="hp")
                for k in range(KT):
                    for h in range(2):
                        nc.tensor.matmul(
                            hw[:, 512 * h : 512 * h + 512],
                            G[:, 16 * k : 16 * k + 16],
                            Wd_r[:, k * HID + 512 * h : k * HID + 512 * h + 512],
                            start=(k == 0),
                            stop=(k == KT - 1),
                        )
                u = up.tile([BS, HID], FP32)
                nc.vector.tensor_add(u[:], hw[:], zt[:])
                s = spp.tile([BS, HID], FP32)
                nc.scalar.activation(s[:], u[:], AF.Tanh)
                for k in range(KT):
                    for h in range(2):
                        nc.tensor.matmul(
                            hp[:, 512 * h : 512 * h + 512],
                            G[:, 16 * k : 16 * k + 16],
                            P_r[:, k * HID + 512 * h : k * HID + 512 * h + 512],
                            start=(k == 0),
                            stop=(k == KT - 1),
                        )
                Hn = hnp.tile([BS, HID], FP32)
                nc.vector.tensor_add(Hn[:], hp[:], s[:])
                Gn = gp.tile([128, 128], FP32R)
                trp = trpp.tile([128, 128], FP32)
                for j in range(KT):
                    nc.tensor.transpose(
                        trp[:, 16 * j : 16 * j + 16],
                        Hn[:, 128 * j : 128 * j + 128],
                        ident[:BS, :BS],
                    )
                    nc.vector.tensor_copy(
                        Gn[:, 16 * j : 16 * j + 16], trp[:, 16 * j : 16 * j + 16]
                    )
                G = Gn

            # ---- final linear: out = dt * (H_T @ D_w.T) + D_b ----
            with tc.tile_pool(name="fin", bufs=1) as fin, tc.tile_pool(
                name="fps", bufs=1, space="PSUM"
            ) as fps:
                po = fps.tile([BS, OUT], FP32)
                for k in range(KT):
                    nc.tensor.matmul(
                        po[:],
                        G[:, 16 * k : 16 * k + 16],
                        Dwt_r[:, OUT * k : OUT * k + OUT],
                        start=(k == 0),
                        stop=(k == KT - 1),
                    )
                ob = fin.tile([BS, OUT], FP32)
                nc.vector.scalar_tensor_tensor(
                    ob[:], po[:], DT, Dbb_sb[:BS, :], ALU.mult, ALU.add
                )
                nc.sync.dma_start(out[:], ob[:])

    nc.finalize()
    return nc


def make_in_maps(x, M_W, M_A, E_w, E_b, D_w, D_b):
    f32 = lambda a: np.ascontiguousarray(np.asarray(a, dtype=np.float32))
    x = f32(x)
    Ewt = f32(np.asarray(E_w, np.float32).T)
    Ebb = f32(np.tile(np.asarray(E_b, np.float32)[None, :], (B, 1)))
    Dwt = f32(np.asarray(D_w, np.float32).T)
    Dbb = f32(np.tile(np.asarray(D_b, np.float32)[None, :], (B, 1)))
    MAc, MWc = f32(M_A), f32(M_W)
    in_maps = []
    for c in range(NCORES):
        in_maps.append(
            {
                "xs": f32(x[:, :, BS * c : BS * (c + 1)]),
                "MA": MAc,
                "MW": MWc,
                "Ewt": Ewt,
                "Ebb": Ebb,
                "Dwt": Dwt,
                "Dbb": Dbb,
            }
        )
    return in_maps


_NC_CACHE = {}


def _get_nc(t_steps=T):
    if t_steps not in _NC_CACHE:
        _NC_CACHE[t_steps] = build(t_steps)
    return _NC_CACHE[t_steps]


def kernel(x, M_W, M_A, E_w, E_b, D_w, D_b):
    nc = _get_nc(T)
    in_maps = make_in_maps(x, M_W, M_A, E_w, E_b, D_w, D_b)
    res = run_bass_kernel_spmd(nc, in_maps, list(range(NCORES)))
    return np.concatenate(
        [res.results[c]["out"] for c in range(NCORES)], axis=0
    ).astype(np.float32)
